# revision 8
# baseline (speedup 1.0000x reference)
"""Trainium2 Bass kernel: AABlock (qkv proj + conditional 2D RoPE + dense softmax
attention + out proj), SPMD across 8 NeuronCores.

Sharding: (batch, head) pairs. Core r handles batch r//4, heads 3*(r%4)..3*(r%4)+2.
Each core computes q,k,v for its heads (transposed flash layout), applies RoPE via
host-precomputed cos/sin multiplier tables, and runs softmax attention without max
subtraction (scores are bounded; verified |scale*S| < 16). Each core then applies
the output projection restricted to its own heads' 192 input dims, producing a
partial [N, C] sum; the host unshard step sums the 4 partials per batch (the
tensor-parallel output reconstruction) and adds proj_b. All matmul/softmax FLOPs
run on device; the host only preprocesses inputs and reduces/gathers outputs.

kernel(**inputs) accepts the FULL unsharded inputs and returns the FULL output.
"""
import os
import sys
import types
import contextlib

import numpy as np

try:  # make concourse importable both in the dev container and standalone
    import concourse.bass as _probe  # noqa: F401
except ImportError:  # pragma: no cover
    sys.path.insert(0, "/opt/trn_rl_repo")

import ml_dtypes
import concourse.bass as bass
import concourse.tile as tile
from concourse import bacc, mybir
from concourse.bass_utils import run_bass_kernel_spmd

BF16 = ml_dtypes.bfloat16
ROPE_FREQ = 100.0
NCORES = 8
B, C, H, HD = 2, 768, 12, 64
HL = 3  # heads per core
SCALE = HD ** -0.5
SHUF_MASK = [(i + 16) % 32 for i in range(32)]

_CACHE = {}
LAST_RESULT = None  # BassKernelResults of the most recent run (for test harness)


# ----------------------------------------------------------------------------- host prep

def _rope_tables(pos2d, rope_mask):
    """A (cos-or-1) and signed-S (sin-or-0) multipliers, [B, N, 64] f32 each.

    q_roped = q * A + shuffle(q) * S, where shuffle swaps 16-blocks within each
    32-block of the head dim (matches DVE stream_shuffle with SHUF_MASK).
    """
    Bn, n, _ = pos2d.shape
    pos = np.asarray(pos2d, dtype=np.float32)
    mask = np.asarray(rope_mask, dtype=bool)
    inv_freq = 1.0 / (ROPE_FREQ ** (np.arange(0, 32, 2, dtype=np.float32) / 32))
    cos = np.empty((Bn, n, 64), np.float32)
    sin = np.empty((Bn, n, 64), np.float32)
    for half, pidx in ((0, 0), (1, 1)):
        ang = pos[:, :, pidx:pidx + 1] * inv_freq[None, None, :]
        c, s = np.cos(ang), np.sin(ang)
        base = half * 32
        cos[:, :, base:base + 16] = c
        cos[:, :, base + 16:base + 32] = c
        sin[:, :, base:base + 16] = -s
        sin[:, :, base + 16:base + 32] = s
    A = np.where(mask[:, :, None], cos, np.float32(1.0))
    S = np.where(mask[:, :, None], sin, np.float32(0.0))
    return A.astype(np.float32), S.astype(np.float32)


def _prep_inputs(x, qkv_w, qkv_b, proj_w, proj_b, pos2d, rope_mask):
    n = x.shape[1]
    aug = bool(np.any(qkv_b))
    nk = 7 if aug else 6

    A, S = _rope_tables(pos2d, rope_mask)

    xts, aqks, sqks = [], [], []
    for b in range(B):
        xt = np.zeros((nk * 128, n), np.float32)
        xt[0:C] = np.asarray(x[b]).T
        if aug:
            xt[C] = 1.0
        xts.append(xt.astype(BF16))
        aqks.append(np.concatenate([A[b].T, A[b].T], axis=0).copy())  # [128, n]
        sqks.append(np.concatenate([S[b].T, S[b].T], axis=0).copy())

    Wq, Wk, Wv = qkv_w[0:C], qkv_w[C:2 * C], qkv_w[2 * C:3 * C]
    bq, bk, bv = qkv_b[0:C], qkv_b[C:2 * C], qkv_b[2 * C:3 * C]

    def wcol(Wm, bvec, h):  # -> [nk*128, 64] (transposed weight block + opt bias row)
        blk = np.zeros((nk * 128, 64), np.float32)
        blk[0:C] = np.asarray(Wm[h * 64:(h + 1) * 64, :]).T
        if aug:
            blk[C] = np.asarray(bvec[h * 64:(h + 1) * 64])
        return blk

    wpT = np.asarray(proj_w).T  # [in=d, out]

    in_maps = []
    for r in range(NCORES):
        b, rank = r // 4, r % 4
        hs = [3 * rank, 3 * rank + 1, 3 * rank + 2]
        wqk = np.concatenate(
            [wcol(Wq, bq, hs[0]), wcol(Wq, bq, hs[1]),
             wcol(Wk, bk, hs[0]), wcol(Wk, bk, hs[1]),
             wcol(Wq, bq, hs[2]), wcol(Wq, bq, hs[2]),
             wcol(Wk, bk, hs[2]), wcol(Wk, bk, hs[2])], axis=1).astype(BF16)
        wv_host = np.concatenate(
            [wcol(Wv, bv, hs[0]), wcol(Wv, bv, hs[1]), wcol(Wv, bv, hs[2])],
            axis=1).astype(BF16)
        wp_host = np.concatenate(
            [wpT[h * 64:(h + 1) * 64] for h in hs], axis=0).astype(BF16)  # [192, 768]
        in_maps.append({
            "xt": xts[b], "aqk": aqks[b], "sqk": sqks[b],
            "wqk": wqk, "wv": wv_host, "wp": np.ascontiguousarray(wp_host),
        })
    return in_maps, nk, n


# ----------------------------------------------------------------------------- device graph

def _build_nc(n, nk):
    dt = mybir.dt
    KT = n // 128       # key-token tiles
    NCH = n // 512      # 512-wide token chunks (qkv phase)
    QH = 1024           # exp granularity / PV psum width along q
    assert n % QH == 0
    NQH = n // QH
    TT = n // 128

    nc = bacc.Bacc("TRN2", target_bir_lowering=False, debug=False,
                   num_devices=NCORES)
    xt = nc.declare_dram_parameter("xt", [nk * 128, n], dt.bfloat16, isOutput=False)
    aqk = nc.declare_dram_parameter("aqk", [128, n], dt.float32, isOutput=False)
    sqk = nc.declare_dram_parameter("sqk", [128, n], dt.float32, isOutput=False)
    wqk = nc.declare_dram_parameter("wqk", [nk * 128, 512], dt.bfloat16, isOutput=False)
    wv = nc.declare_dram_parameter("wv", [nk * 128, 192], dt.bfloat16, isOutput=False)
    wp = nc.declare_dram_parameter("wp", [192, 768], dt.bfloat16, isOutput=False)
    out = nc.declare_dram_parameter("out", [n, 768], dt.float32, isOutput=True)

    rs_dram = [nc.dram_tensor(f"rs_dram{p}", [1, n], dt.float32) for p in range(HL)]

    with tile.TileContext(nc) as tc, contextlib.ExitStack() as ctx:
        P1 = ctx.enter_context(tc.tile_pool(name="persist", bufs=1))
        RP = ctx.enter_context(tc.tile_pool(name="rope", bufs=2))
        ES = ctx.enter_context(tc.tile_pool(name="es", bufs=4))
        EV = ctx.enter_context(tc.tile_pool(name="evict", bufs=2))
        PS = ctx.enter_context(tc.tile_pool(name="ps", bufs=2, space="PSUM"))
        PSO = ctx.enter_context(tc.tile_pool(name="pso", bufs=2, space="PSUM"))

        # ---- persistent SBUF tensors + input DMAs (weights first: first matmuls
        # need wqk[0] + xt[0] only)
        wqk_sb = P1.tile([128, nk, 512], dt.bfloat16)
        xt_sb = P1.tile([128, nk, n], dt.bfloat16)
        for k in range(nk):
            nc.sync.dma_start(out=wqk_sb[:, k, :], in_=wqk[k * 128:(k + 1) * 128, :])
            nc.sync.dma_start(out=xt_sb[:, k, :], in_=xt[k * 128:(k + 1) * 128, :])
        aqk_sb = P1.tile([128, n], dt.float32)
        nc.sync.dma_start(out=aqk_sb, in_=aqk[:, :])
        sqk_sb = P1.tile([128, n], dt.float32)
        nc.sync.dma_start(out=sqk_sb, in_=sqk[:, :])
        wv_sb = P1.tile([128, nk, 192], dt.bfloat16)
        for k in range(nk):
            nc.sync.dma_start(out=wv_sb[:, k, :], in_=wv[k * 128:(k + 1) * 128, :])
        wp_sb = P1.tile([64, HL, 768], dt.bfloat16)
        for p in range(HL):
            nc.sync.dma_start(out=wp_sb[:, p, :], in_=wp[p * 64:(p + 1) * 64, :])

        qT_sb = P1.tile([128, 2, n], dt.bfloat16)
        kT_sb = P1.tile([128, 2, n], dt.bfloat16)
        v_sb = P1.tile([128, KT, HL, 66], dt.bfloat16)
        attnT_sb = P1.tile([64, HL, n], dt.bfloat16)
        oacc_sb = P1.tile([128, TT, 768], dt.float32)

        nc.vector.memset(v_sb[:, :, :, 64:65], 1.0)

        # ---- qk projection m-tile + RoPE (emitted on demand)
        def qk_mtile(m, ch):
            dest = qT_sb if m % 2 == 0 else kT_sb
            j = m // 2
            ps = PS.tile([128, QH], dt.float32, tag="s")
            for k in range(nk):
                nc.tensor.matmul(ps[:, 0:512],
                                 lhsT=wqk_sb[:, k, m * 128:(m + 1) * 128],
                                 rhs=xt_sb[:, k, ch * 512:(ch + 1) * 512],
                                 start=(k == 0), stop=(k == nk - 1))
            sl = slice(ch * 512, (ch + 1) * 512)
            raw = RP.tile([128, 512], dt.float32, tag="raw")
            nc.vector.tensor_copy(out=raw, in_=ps[:, 0:512])
            rot = RP.tile([128, 512], dt.float32, tag="rot")
            nc.vector.stream_shuffle(rot, raw, SHUF_MASK)
            t1 = RP.tile([128, 512], dt.float32, tag="t1")
            nc.vector.tensor_mul(t1, raw, aqk_sb[:, sl])
            t2 = RP.tile([128, 512], dt.float32, tag="t2")
            nc.vector.tensor_mul(t2, rot, sqk_sb[:, sl])
            nc.vector.tensor_add(dest[:, j, sl], t1, t2)

        def v_mtile(tt):
            psv = PS.tile([128, QH], dt.float32, tag="s")
            for k in range(nk):
                nc.tensor.matmul(psv[:, 0:192],
                                 lhsT=xt_sb[:, k, tt * 128:(tt + 1) * 128],
                                 rhs=wv_sb[:, k, :],
                                 start=(k == 0), stop=(k == nk - 1))
            nc.vector.tensor_copy(
                out=v_sb[:, tt, :, 0:64],
                in_=psv[:, 0:192].rearrange("p (h d) -> p h d", h=HL))

        def evict(hl, qh, ps_o):
            qsl = slice(qh * QH, (qh + 1) * QH)
            rs65 = EV.tile([65, QH], dt.float32, tag="rs")
            nc.vector.reciprocal(out=rs65[64:65, :], in_=ps_o[64:65, :])
            nc.sync.dma_start(out=rs_dram[hl][0:1, qsl], in_=rs65[64:65, :])
            rs_b = EV.tile([64, QH], dt.float32, tag="rsb")
            nc.gpsimd.dma_start(out=rs_b,
                                in_=rs_dram[hl][0:1, qsl].to_broadcast((64, QH)))
            nc.vector.tensor_mul(attnT_sb[:, hl, qsl], ps_o[0:64, :], rs_b)

        def s_mm(hl, kt, qh, ps_s):
            j, hb = ((0, 0), (0, 64), (1, 0))[hl]
            if hl == 2 and kt % 2 == 1:
                hb = 64  # duplicate copy of head 2 at partitions 64-127
            for sub in range(QH // 512):
                qoff = qh * QH + sub * 512
                nc.tensor.matmul(
                    ps_s[:, sub * 512:(sub + 1) * 512],
                    lhsT=kT_sb[hb:hb + 64, j, kt * 128:(kt + 1) * 128],
                    rhs=qT_sb[hb:hb + 64, j, qoff:qoff + 512],
                    start=True, stop=True)

        def exp_op(ps_s):
            es = ES.tile([128, QH], dt.bfloat16, tag="es")
            nc.scalar.activation(out=es, in_=ps_s,
                                 func=mybir.ActivationFunctionType.Exp,
                                 scale=float(SCALE))
            return es

        def pv_mm(hl, kt, ps_o, es, first, last):
            for sub in range(QH // 512):
                nc.tensor.matmul(
                    ps_o[:, sub * 512:(sub + 1) * 512],
                    lhsT=v_sb[:, kt, hl, 0:65],
                    rhs=es[:, sub * 512:(sub + 1) * 512],
                    start=first, stop=last,
                    skip_group_check=True)

        def proj_sweep(tt, heads, accumulate):
            psp = PS.tile([128, QH], dt.float32, tag="s")
            for i, hl in enumerate(heads):
                lhsT = attnT_sb[:, hl, tt * 128:(tt + 1) * 128]
                nc.tensor.matmul(psp[:, 0:512], lhsT=lhsT, rhs=wp_sb[:, hl, 0:512],
                                 start=(i == 0), stop=(i == len(heads) - 1),
                                 skip_group_check=True)
                nc.tensor.matmul(psp[:, 512:768], lhsT=lhsT,
                                 rhs=wp_sb[:, hl, 512:768],
                                 start=(i == 0), stop=(i == len(heads) - 1),
                                 skip_group_check=True)
            if not accumulate:
                nc.vector.tensor_copy(out=oacc_sb[:, tt, :], in_=psp[:, 0:768])
            else:
                ob = EV.tile([128, 768], dt.float32, tag="ob")
                nc.vector.tensor_add(ob, psp[:, 0:768], oacc_sb[:, tt, :])
                nc.sync.dma_start(out=out[tt * 128:(tt + 1) * 128, :], in_=ob)

        # ---- phase A: q/k for heads 0,1 (m-tiles 0,1)
        for m in (0, 1):
            for ch in range(NCH):
                qk_mtile(m, ch)

        # ---- phase B: heads 0+1 attention, interleaved for PE row-group packing.
        # V tiles are produced lazily in the qh=0 sweep; m-tiles 2,3 (head 2 q/k)
        # are sprinkled into the qh=1 sweep.
        sprinkle = []
        for ch in range(NCH):
            sprinkle.append((2, ch))
            sprinkle.append((3, ch))
        for qh in range(NQH):
            ps_oA = PSO.tile([65, QH], dt.float32, tag="o")
            ps_oB = PSO.tile([65, QH], dt.float32, tag="o")
            for kt in range(KT):
                ps_sA = PS.tile([128, QH], dt.float32, tag="s")
                s_mm(0, kt, qh, ps_sA)
                ps_sB = PS.tile([128, QH], dt.float32, tag="s")
                s_mm(1, kt, qh, ps_sB)
                if qh == 0:
                    v_mtile(kt)
                elif os.environ.get("AAB_SPRINKLE") and kt < len(sprinkle):
                    qk_mtile(*sprinkle[kt])
                esA = exp_op(ps_sA)
                esB = exp_op(ps_sB)
                pv_mm(0, kt, ps_oA, esA, kt == 0, kt == KT - 1)
                pv_mm(1, kt, ps_oB, esB, kt == 0, kt == KT - 1)
            evict(0, qh, ps_oA)
            evict(1, qh, ps_oB)

        if not os.environ.get("AAB_SPRINKLE"):
            for m, ch in sprinkle:
                qk_mtile(m, ch)

        # ---- phase C: head 2 attention (kt pairs packed via its duplicate q/k
        # copy at partitions 64-127), overlapped with proj sweep A (heads 0,1).
        projA = list(range(TT))
        for qh in range(NQH):
            ps_oA = PSO.tile([65, QH], dt.float32, tag="o")
            for kt2 in range(KT // 2):
                ps_sA = PS.tile([128, QH], dt.float32, tag="s")
                s_mm(2, 2 * kt2, qh, ps_sA)
                ps_sB = PS.tile([128, QH], dt.float32, tag="s")
                s_mm(2, 2 * kt2 + 1, qh, ps_sB)
                if projA:
                    proj_sweep(projA.pop(0), (0, 1), accumulate=False)
                esA = exp_op(ps_sA)
                esB = exp_op(ps_sB)
                pv_mm(2, 2 * kt2, ps_oA, esA, kt2 == 0, False)
                pv_mm(2, 2 * kt2 + 1, ps_oA, esB, False, kt2 == KT // 2 - 1)
            evict(2, qh, ps_oA)

        # ---- phase D: remaining proj sweep A tiles (if any), then sweep B (head 2)
        for tt in projA:
            proj_sweep(tt, (0, 1), accumulate=False)
        for tt in range(TT):
            proj_sweep(tt, (2,), accumulate=True)

        if os.environ.get("AAB_DEBUG"):
            dbg_qT = nc.declare_dram_parameter("dbg_qT", [128, 2 * n], dt.bfloat16,
                                               isOutput=True)
            dbg_kT = nc.declare_dram_parameter("dbg_kT", [128, 2 * n], dt.bfloat16,
                                               isOutput=True)
            dbg_at = nc.declare_dram_parameter("dbg_at", [64, HL * n], dt.bfloat16,
                                               isOutput=True)
            dbg_oa = nc.declare_dram_parameter("dbg_oa", [128, TT * 768], dt.float32,
                                               isOutput=True)
            dbg_v = nc.declare_dram_parameter("dbg_v", [128, KT * HL * 66],
                                              dt.bfloat16, isOutput=True)
            nc.sync.dma_start(out=dbg_qT[:, :], in_=qT_sb[:, :, :])
            nc.sync.dma_start(out=dbg_kT[:, :], in_=kT_sb[:, :, :])
            nc.sync.dma_start(out=dbg_at[:, :], in_=attnT_sb[:, :, :])
            nc.sync.dma_start(out=dbg_oa[:, :], in_=oacc_sb[:, :, :])
            nc.sync.dma_start(out=dbg_v[:, :], in_=v_sb[:, :, :, :])

    nc.compile()
    return nc


def _get_nc(n, nk):
    key = (n, nk)
    if key not in _CACHE:
        _CACHE[key] = _build_nc(n, nk)
    return _CACHE[key]


# ----------------------------------------------------------------------------- profiling shim

def _install_prof_shim():
    try:
        import antenv.axon_hooks  # noqa: F401
        return
    except ImportError:
        pass
    mod = types.ModuleType("antenv.axon_hooks")
    _store = {}
    mod.set_axon_ntff_profile_hook = lambda h: _store.__setitem__("h", h)
    mod.get_axon_ntff_profile_hook = lambda: _store.get("h")
    sys.modules["antenv.axon_hooks"] = mod
    import antenv
    antenv.axon_hooks = mod
    try:
        from trn_agent_boot.trn_boot import _ntff_profile_via_ctypes
        mod.set_axon_ntff_profile_hook(
            _ntff_profile_via_ctypes("/opt/axon/libaxon_pjrt.so"))
        import concourse.bass_utils as bu
        bu.upload_artifacts = lambda tmpdir: str(tmpdir)
    except Exception:
        pass


# ----------------------------------------------------------------------------- entry point

def kernel(x, qkv_w, qkv_b, proj_w, proj_b, pos2d, rope_mask):
    global LAST_RESULT
    x = np.asarray(x, dtype=np.float32)
    qkv_w = np.asarray(qkv_w, dtype=np.float32)
    qkv_b = np.asarray(qkv_b, dtype=np.float32)
    proj_w = np.asarray(proj_w, dtype=np.float32)
    proj_b = np.asarray(proj_b, dtype=np.float32)

    in_maps, nk, n = _prep_inputs(x, qkv_w, qkv_b, proj_w, proj_b, pos2d, rope_mask)
    nc = _get_nc(n, nk)

    trace = bool(os.environ.get("AAB_TRACE"))
    if trace:
        _install_prof_shim()
    res = run_bass_kernel_spmd(nc, in_maps, list(range(NCORES)), trace=trace)
    LAST_RESULT = res

    # unshard: each core returned a partial [n, C] projection (its 3 heads);
    # sum the 4 partials per batch and add proj_b.
    outs = np.empty((B, n, C), np.float32)
    for b in range(B):
        acc = res.results[4 * b]["out"].astype(np.float32)
        for j in range(1, 4):
            acc = acc + res.results[4 * b + j]["out"]
        outs[b] = acc
    if np.any(proj_b):
        outs += proj_b[None, None, :]
    return outs


# revision 9
# speedup vs baseline: 2.4763x; 2.4763x over previous
"""Trainium2 Bass kernel: AABlock (qkv proj + conditional 2D RoPE + dense softmax
attention + out proj), SPMD across 8 NeuronCores.

Sharding: (batch, head) pairs. Core r handles batch r//4, heads 3*(r%4)..3*(r%4)+2.
Each core computes q,k,v for its heads (transposed flash layout), applies RoPE via
host-precomputed cos/sin multiplier tables, and runs softmax attention without max
subtraction (scores are bounded; verified |scale*S| < 16). Each core then applies
the output projection restricted to its own heads' 192 input dims, producing a
partial [N, C] sum; the host unshard step sums the 4 partials per batch (the
tensor-parallel output reconstruction) and adds proj_b. All matmul/softmax FLOPs
run on device; the host only preprocesses inputs and reduces/gathers outputs.

kernel(**inputs) accepts the FULL unsharded inputs and returns the FULL output.
"""
import os
import sys
import types
import contextlib

import numpy as np

try:  # make concourse importable both in the dev container and standalone
    import concourse.bass as _probe  # noqa: F401
except ImportError:  # pragma: no cover
    sys.path.insert(0, "/opt/trn_rl_repo")

import ml_dtypes
import concourse.bass as bass
import concourse.tile as tile
from concourse import bacc, mybir
from concourse.bass_utils import run_bass_kernel_spmd

BF16 = ml_dtypes.bfloat16
ROPE_FREQ = 100.0
NCORES = 8
B, C, H, HD = 2, 768, 12, 64
HL = 3  # heads per core
SCALE = HD ** -0.5
SHUF_MASK = [(i + 16) % 32 for i in range(32)]

_CACHE = {}
LAST_RESULT = None  # BassKernelResults of the most recent run (for test harness)


# ----------------------------------------------------------------------------- host prep

def _rope_tables(pos2d, rope_mask):
    """A (cos-or-1) and signed-S (sin-or-0) multipliers, [B, N, 64] f32 each.

    q_roped = q * A + shuffle(q) * S, where shuffle swaps 16-blocks within each
    32-block of the head dim (matches DVE stream_shuffle with SHUF_MASK).
    """
    Bn, n, _ = pos2d.shape
    pos = np.asarray(pos2d, dtype=np.float32)
    mask = np.asarray(rope_mask, dtype=bool)
    inv_freq = 1.0 / (ROPE_FREQ ** (np.arange(0, 32, 2, dtype=np.float32) / 32))
    cos = np.empty((Bn, n, 64), np.float32)
    sin = np.empty((Bn, n, 64), np.float32)
    for half, pidx in ((0, 0), (1, 1)):
        ang = pos[:, :, pidx:pidx + 1] * inv_freq[None, None, :]
        c, s = np.cos(ang), np.sin(ang)
        base = half * 32
        cos[:, :, base:base + 16] = c
        cos[:, :, base + 16:base + 32] = c
        sin[:, :, base:base + 16] = -s
        sin[:, :, base + 16:base + 32] = s
    A = np.where(mask[:, :, None], cos, np.float32(1.0))
    S = np.where(mask[:, :, None], sin, np.float32(0.0))
    return A.astype(np.float32), S.astype(np.float32)


def _prep_inputs(x, qkv_w, qkv_b, proj_w, proj_b, pos2d, rope_mask):
    n = x.shape[1]
    aug = bool(np.any(qkv_b))
    nk = 7 if aug else 6

    A, S = _rope_tables(pos2d, rope_mask)

    xts, aqks, sqks = [], [], []
    for b in range(B):
        xt = np.zeros((nk * 128, n), np.float32)
        xt[0:C] = np.asarray(x[b]).T
        if aug:
            xt[C] = 1.0
        xts.append(xt.astype(BF16))
        aqks.append(np.concatenate([A[b].T, A[b].T], axis=0).copy())  # [128, n]
        sqks.append(np.concatenate([S[b].T, S[b].T], axis=0).copy())

    Wq, Wk, Wv = qkv_w[0:C], qkv_w[C:2 * C], qkv_w[2 * C:3 * C]
    bq, bk, bv = qkv_b[0:C], qkv_b[C:2 * C], qkv_b[2 * C:3 * C]

    def wcol(Wm, bvec, h):  # -> [nk*128, 64] (transposed weight block + opt bias row)
        blk = np.zeros((nk * 128, 64), np.float32)
        blk[0:C] = np.asarray(Wm[h * 64:(h + 1) * 64, :]).T
        if aug:
            blk[C] = np.asarray(bvec[h * 64:(h + 1) * 64])
        return blk

    wpT = np.asarray(proj_w).T  # [in=d, out]

    in_maps = []
    for r in range(NCORES):
        b, rank = r // 4, r % 4
        hs = [3 * rank, 3 * rank + 1, 3 * rank + 2]
        wqk = np.concatenate(
            [wcol(Wq, bq, hs[0]), wcol(Wq, bq, hs[1]),
             wcol(Wk, bk, hs[0]), wcol(Wk, bk, hs[1]),
             wcol(Wq, bq, hs[2]), wcol(Wq, bq, hs[2]),
             wcol(Wk, bk, hs[2]), wcol(Wk, bk, hs[2])], axis=1).astype(BF16)
        wv_host = np.concatenate(
            [wcol(Wv, bv, hs[0]), wcol(Wv, bv, hs[1]), wcol(Wv, bv, hs[2])],
            axis=1).astype(BF16)
        wp_host = np.concatenate(
            [wpT[h * 64:(h + 1) * 64] for h in hs], axis=0).astype(BF16)  # [192, 768]
        in_maps.append({
            "xt": xts[b], "aqk": aqks[b], "sqk": sqks[b],
            "wqk": wqk, "wv": wv_host, "wp": np.ascontiguousarray(wp_host),
        })
    return in_maps, nk, n


# ----------------------------------------------------------------------------- device graph

def _build_nc(n, nk):
    dt = mybir.dt
    KT = n // 128       # key-token tiles
    NCH = n // 512      # 512-wide token chunks (qkv phase)
    QH = 1024           # exp granularity / PV psum width along q
    assert n % QH == 0
    NQH = n // QH
    TT = n // 128

    nc = bacc.Bacc("TRN2", target_bir_lowering=False, debug=False,
                   num_devices=NCORES)
    xt = nc.declare_dram_parameter("xt", [nk * 128, n], dt.bfloat16, isOutput=False)
    aqk = nc.declare_dram_parameter("aqk", [128, n], dt.float32, isOutput=False)
    sqk = nc.declare_dram_parameter("sqk", [128, n], dt.float32, isOutput=False)
    wqk = nc.declare_dram_parameter("wqk", [nk * 128, 512], dt.bfloat16, isOutput=False)
    wv = nc.declare_dram_parameter("wv", [nk * 128, 192], dt.bfloat16, isOutput=False)
    wp = nc.declare_dram_parameter("wp", [192, 768], dt.bfloat16, isOutput=False)
    out = nc.declare_dram_parameter("out", [n, 768], dt.float32, isOutput=True)

    rs_dram = [nc.dram_tensor(f"rs_dram{p}", [1, n], dt.float32) for p in range(HL)]

    with tile.TileContext(nc) as tc, contextlib.ExitStack() as ctx:
        P1 = ctx.enter_context(tc.tile_pool(name="persist", bufs=1))
        RP = ctx.enter_context(tc.tile_pool(name="rope", bufs=2))
        ES = ctx.enter_context(tc.tile_pool(name="es", bufs=4))
        EV = ctx.enter_context(tc.tile_pool(name="evict", bufs=2))
        PS = ctx.enter_context(tc.tile_pool(name="ps", bufs=2, space="PSUM"))
        PSO = ctx.enter_context(tc.tile_pool(name="pso", bufs=2, space="PSUM"))

        # ---- persistent SBUF tensors + input DMAs (weights first: first matmuls
        # need wqk[0] + xt[0] only)
        wqk_sb = P1.tile([128, nk, 512], dt.bfloat16)
        xt_sb = P1.tile([128, nk, n], dt.bfloat16)
        for k in range(nk):
            nc.sync.dma_start(out=wqk_sb[:, k, :], in_=wqk[k * 128:(k + 1) * 128, :])
            nc.sync.dma_start(out=xt_sb[:, k, :], in_=xt[k * 128:(k + 1) * 128, :])
        aqk_sb = P1.tile([128, n], dt.float32)
        nc.sync.dma_start(out=aqk_sb, in_=aqk[:, :])
        sqk_sb = P1.tile([128, n], dt.float32)
        nc.sync.dma_start(out=sqk_sb, in_=sqk[:, :])
        wv_sb = P1.tile([128, nk, 192], dt.bfloat16)
        for k in range(nk):
            nc.sync.dma_start(out=wv_sb[:, k, :], in_=wv[k * 128:(k + 1) * 128, :])
        wp_sb = P1.tile([64, HL, 768], dt.bfloat16)
        for p in range(HL):
            nc.sync.dma_start(out=wp_sb[:, p, :], in_=wp[p * 64:(p + 1) * 64, :])

        qT_sb = P1.tile([128, 2, n], dt.bfloat16)
        kT_sb = P1.tile([128, 2, n], dt.bfloat16)
        v_sb = P1.tile([128, KT, HL, 66], dt.bfloat16)
        attnT_sb = P1.tile([64, HL, n], dt.bfloat16)
        oacc_sb = P1.tile([128, TT, 768], dt.float32)

        nc.vector.memset(v_sb[:, :, :, 64:65], 1.0)

        # ---- qk projection m-tile + RoPE (emitted on demand)
        def qk_mtile(m, ch):
            dest = qT_sb if m % 2 == 0 else kT_sb
            j = m // 2
            ps = PS.tile([128, QH], dt.float32, tag="s")
            for k in range(nk):
                nc.tensor.matmul(ps[:, 0:512],
                                 lhsT=wqk_sb[:, k, m * 128:(m + 1) * 128],
                                 rhs=xt_sb[:, k, ch * 512:(ch + 1) * 512],
                                 start=(k == 0), stop=(k == nk - 1))
            sl = slice(ch * 512, (ch + 1) * 512)
            rot = RP.tile([128, 512], dt.float32, tag="rot")
            nc.vector.stream_shuffle(rot, ps[:, 0:512], SHUF_MASK)
            t1 = RP.tile([128, 512], dt.float32, tag="t1")
            nc.vector.tensor_mul(t1, ps[:, 0:512], aqk_sb[:, sl])
            t2 = RP.tile([128, 512], dt.float32, tag="t2")
            nc.gpsimd.tensor_tensor(t2, rot, sqk_sb[:, sl], mybir.AluOpType.mult)
            nc.vector.tensor_add(dest[:, j, sl], t1, t2)

        def v_mtile(tt):
            psv = PS.tile([128, QH], dt.float32, tag="s")
            for k in range(nk):
                nc.tensor.matmul(psv[:, 0:192],
                                 lhsT=xt_sb[:, k, tt * 128:(tt + 1) * 128],
                                 rhs=wv_sb[:, k, :],
                                 start=(k == 0), stop=(k == nk - 1))
            nc.vector.tensor_copy(
                out=v_sb[:, tt, :, 0:64],
                in_=psv[:, 0:192].rearrange("p (h d) -> p h d", h=HL))

        def evict(hl, qh, ps_o):
            qsl = slice(qh * QH, (qh + 1) * QH)
            rs65 = EV.tile([65, QH], dt.float32, tag="rs")
            nc.vector.reciprocal_approx_fast(out=rs65[64:65, :], in_=ps_o[64:65, :])
            nc.sync.dma_start(out=rs_dram[hl][0:1, qsl], in_=rs65[64:65, :])
            rs_b = EV.tile([64, QH], dt.float32, tag="rsb")
            nc.gpsimd.dma_start(out=rs_b,
                                in_=rs_dram[hl][0:1, qsl].to_broadcast((64, QH)))
            nc.vector.tensor_mul(attnT_sb[:, hl, qsl], ps_o[0:64, :], rs_b)

        def s_mm(hl, kt, qh, ps_s):
            j, hb = ((0, 0), (0, 64), (1, 0))[hl]
            if hl == 2 and kt % 2 == 1:
                hb = 64  # duplicate copy of head 2 at partitions 64-127
            for sub in range(QH // 512):
                qoff = qh * QH + sub * 512
                nc.tensor.matmul(
                    ps_s[:, sub * 512:(sub + 1) * 512],
                    lhsT=kT_sb[hb:hb + 64, j, kt * 128:(kt + 1) * 128],
                    rhs=qT_sb[hb:hb + 64, j, qoff:qoff + 512],
                    start=True, stop=True)

        def exp_op(ps_s):
            es = ES.tile([128, QH], dt.bfloat16, tag="es")
            nc.scalar.activation(out=es, in_=ps_s,
                                 func=mybir.ActivationFunctionType.Exp,
                                 scale=float(SCALE))
            return es

        def pv_mm(hl, kt, ps_o, es, first, last):
            for sub in range(QH // 512):
                nc.tensor.matmul(
                    ps_o[:, sub * 512:(sub + 1) * 512],
                    lhsT=v_sb[:, kt, hl, 0:65],
                    rhs=es[:, sub * 512:(sub + 1) * 512],
                    start=first, stop=last,
                    skip_group_check=True)

        def proj_sweep(tt, heads, accumulate):
            psp = PS.tile([128, QH], dt.float32, tag="s")
            for i, hl in enumerate(heads):
                lhsT = attnT_sb[:, hl, tt * 128:(tt + 1) * 128]
                nc.tensor.matmul(psp[:, 0:512], lhsT=lhsT, rhs=wp_sb[:, hl, 0:512],
                                 start=(i == 0), stop=(i == len(heads) - 1),
                                 skip_group_check=True)
                nc.tensor.matmul(psp[:, 512:768], lhsT=lhsT,
                                 rhs=wp_sb[:, hl, 512:768],
                                 start=(i == 0), stop=(i == len(heads) - 1),
                                 skip_group_check=True)
            if not accumulate:
                nc.vector.tensor_copy(out=oacc_sb[:, tt, :], in_=psp[:, 0:768])
            else:
                ob = EV.tile([128, 768], dt.float32, tag="ob")
                nc.vector.tensor_add(ob, psp[:, 0:768], oacc_sb[:, tt, :])
                nc.sync.dma_start(out=out[tt * 128:(tt + 1) * 128, :], in_=ob)

        # ---- phase A: q/k for heads 0,1 (m-tiles 0,1)
        for m in (0, 1):
            for ch in range(NCH):
                qk_mtile(m, ch)

        # ---- phase B: heads 0+1 attention, interleaved for PE row-group packing.
        # V tiles are produced lazily in the qh=0 sweep; m-tiles 2,3 (head 2 q/k)
        # are sprinkled into the qh=1 sweep.
        sprinkle = []
        for ch in range(NCH):
            sprinkle.append((2, ch))
            sprinkle.append((3, ch))
        for qh in range(NQH):
            ps_oA = PSO.tile([65, QH], dt.float32, tag="o")
            ps_oB = PSO.tile([65, QH], dt.float32, tag="o")
            for kt in range(KT):
                ps_sA = PS.tile([128, QH], dt.float32, tag="s")
                s_mm(0, kt, qh, ps_sA)
                ps_sB = PS.tile([128, QH], dt.float32, tag="s")
                s_mm(1, kt, qh, ps_sB)
                if qh == 0:
                    v_mtile(kt)
                esA = exp_op(ps_sA)
                esB = exp_op(ps_sB)
                pv_mm(0, kt, ps_oA, esA, kt == 0, kt == KT - 1)
                pv_mm(1, kt, ps_oB, esB, kt == 0, kt == KT - 1)
            if qh == 0:
                # head-2 q/k projections: PE does these while the qh=0
                # evictions + qh=1 warmup proceed; ropes overlap qh=1 exps
                for m, ch in sprinkle:
                    qk_mtile(m, ch)
            evict(0, qh, ps_oA)
            evict(1, qh, ps_oB)

        # ---- phase C: head 2 attention (kt pairs packed via its duplicate q/k
        # copy at partitions 64-127), overlapped with proj sweep A (heads 0,1).
        projA = list(range(TT))
        for qh in range(NQH):
            ps_oA = PSO.tile([65, QH], dt.float32, tag="o")
            for kt2 in range(KT // 2):
                ps_sA = PS.tile([128, QH], dt.float32, tag="s")
                s_mm(2, 2 * kt2, qh, ps_sA)
                ps_sB = PS.tile([128, QH], dt.float32, tag="s")
                s_mm(2, 2 * kt2 + 1, qh, ps_sB)
                if projA:
                    proj_sweep(projA.pop(0), (0, 1), accumulate=False)
                esA = exp_op(ps_sA)
                esB = exp_op(ps_sB)
                pv_mm(2, 2 * kt2, ps_oA, esA, kt2 == 0, False)
                pv_mm(2, 2 * kt2 + 1, ps_oA, esB, False, kt2 == KT // 2 - 1)
            evict(2, qh, ps_oA)

        # ---- phase D: remaining proj sweep A tiles (if any), then sweep B (head 2)
        for tt in projA:
            proj_sweep(tt, (0, 1), accumulate=False)
        for tt in range(TT):
            proj_sweep(tt, (2,), accumulate=True)

        if os.environ.get("AAB_DEBUG"):
            dbg_qT = nc.declare_dram_parameter("dbg_qT", [128, 2 * n], dt.bfloat16,
                                               isOutput=True)
            dbg_kT = nc.declare_dram_parameter("dbg_kT", [128, 2 * n], dt.bfloat16,
                                               isOutput=True)
            dbg_at = nc.declare_dram_parameter("dbg_at", [64, HL * n], dt.bfloat16,
                                               isOutput=True)
            dbg_oa = nc.declare_dram_parameter("dbg_oa", [128, TT * 768], dt.float32,
                                               isOutput=True)
            dbg_v = nc.declare_dram_parameter("dbg_v", [128, KT * HL * 66],
                                              dt.bfloat16, isOutput=True)
            nc.sync.dma_start(out=dbg_qT[:, :], in_=qT_sb[:, :, :])
            nc.sync.dma_start(out=dbg_kT[:, :], in_=kT_sb[:, :, :])
            nc.sync.dma_start(out=dbg_at[:, :], in_=attnT_sb[:, :, :])
            nc.sync.dma_start(out=dbg_oa[:, :], in_=oacc_sb[:, :, :])
            nc.sync.dma_start(out=dbg_v[:, :], in_=v_sb[:, :, :, :])

    nc.compile()
    return nc


def _get_nc(n, nk):
    key = (n, nk)
    if key not in _CACHE:
        _CACHE[key] = _build_nc(n, nk)
    return _CACHE[key]


# ----------------------------------------------------------------------------- profiling shim

def _install_prof_shim():
    try:
        import antenv.axon_hooks  # noqa: F401
        return
    except ImportError:
        pass
    mod = types.ModuleType("antenv.axon_hooks")
    _store = {}
    mod.set_axon_ntff_profile_hook = lambda h: _store.__setitem__("h", h)
    mod.get_axon_ntff_profile_hook = lambda: _store.get("h")
    sys.modules["antenv.axon_hooks"] = mod
    import antenv
    antenv.axon_hooks = mod
    try:
        from trn_agent_boot.trn_boot import _ntff_profile_via_ctypes
        mod.set_axon_ntff_profile_hook(
            _ntff_profile_via_ctypes("/opt/axon/libaxon_pjrt.so"))
        import concourse.bass_utils as bu
        bu.upload_artifacts = lambda tmpdir: str(tmpdir)
    except Exception:
        pass


# ----------------------------------------------------------------------------- entry point

def kernel(x, qkv_w, qkv_b, proj_w, proj_b, pos2d, rope_mask):
    global LAST_RESULT
    x = np.asarray(x, dtype=np.float32)
    qkv_w = np.asarray(qkv_w, dtype=np.float32)
    qkv_b = np.asarray(qkv_b, dtype=np.float32)
    proj_w = np.asarray(proj_w, dtype=np.float32)
    proj_b = np.asarray(proj_b, dtype=np.float32)

    in_maps, nk, n = _prep_inputs(x, qkv_w, qkv_b, proj_w, proj_b, pos2d, rope_mask)
    nc = _get_nc(n, nk)

    trace = bool(os.environ.get("AAB_TRACE"))
    if trace:
        _install_prof_shim()
    res = run_bass_kernel_spmd(nc, in_maps, list(range(NCORES)), trace=trace)
    LAST_RESULT = res

    # unshard: each core returned a partial [n, C] projection (its 3 heads);
    # sum the 4 partials per batch and add proj_b.
    outs = np.empty((B, n, C), np.float32)
    for b in range(B):
        acc = res.results[4 * b]["out"].astype(np.float32)
        for j in range(1, 4):
            acc = acc + res.results[4 * b + j]["out"]
        outs[b] = acc
    if np.any(proj_b):
        outs += proj_b[None, None, :]
    return outs
